# revision 5
# baseline (speedup 1.0000x reference)
"""Block-local (KeOps-style) attention kernel for 8 TRN2 NeuronCores.

Problem: B=4, H=16, T=8192, D=64; queries in blocks of 256 attend to a
512-key window starting 128 tokens before the block (clamped to [0, T)).

Sharding: the 64 batch*head pairs are split across 8 cores (8 heads per
core, no cross-core communication). On-core, heads are processed in
pairs: two heads' 64 d-rows are stacked onto the 128 SBUF partitions so
DMA runs at full width and the two heads' QK^T matmuls row-pack onto
disjoint halves of the PE array.

Per (pair, T-half, block) the kernel computes, fully on-chip:
  scoresT[k, q] = K_chunk^T-stationary @ Q  (float32r matmuls, keys on
    partitions, 4 chunks of 128 keys; out-of-range chunks are skipped
    entirely which is exactly the reference's -inf mask)
  eT = exp(0.125 * scoresT)                 (one ScalarE pass per head)
  outT[65, 256] = [V | 1]^T-stationary @ eT (row 64 = softmax sums)
  out = outT[0:64] * broadcast(1/sums)      (DVE recip, PE ones-matmul
                                             broadcast, DVE multiply)

The output leaves the chip transposed ([d, t] per head); the host
transposes it back. All host-side work is layout/gather only.
"""

import numpy as np

B, H, T, D = 4, 16, 8192, 64
NCORES = 8
HPC = 8          # heads per core
NPAIR = HPC // 2
HALF = T // 2    # 4096
KT_W = HALF + 128  # key tile width incl. halo
NCH = 33         # value key-chunks per half (32 + 1 halo)
VW = D + 1       # value row width incl. ones column
NBLK = 16        # query blocks per half (block = 256 queries)

TRACE = False
LAST_EXEC_NS = None
LAST_RESULTS = None


def _build_nc():
    import concourse.bacc as bacc
    import concourse.mybir as mybir
    from concourse.tile import TileContext

    f32 = mybir.dt.float32
    f32r = mybir.dt.float32r
    Exp = mybir.ActivationFunctionType.Exp

    nc = bacc.Bacc("TRN2", target_bir_lowering=False, debug=False)
    qt2 = nc.dram_tensor("qt2", [NPAIR, 2, 128, HALF], f32r, kind="ExternalInput")
    kt2 = nc.dram_tensor("kt2", [NPAIR, 2, 128, KT_W], f32r, kind="ExternalInput")
    v1 = nc.dram_tensor("v1", [HPC, 2, 128, NCH * VW], f32r, kind="ExternalInput")
    out2 = nc.dram_tensor("out2", [NPAIR, 128, T], f32, kind="ExternalOutput")

    with TileContext(nc) as tc:
        with (
            tc.tile_pool(name="const", bufs=1) as cpool,
            tc.tile_pool(name="io", bufs=2) as iop,
            tc.tile_pool(name="et", bufs=2) as etp,
            tc.tile_pool(name="outp", bufs=4) as outp,
            tc.tile_pool(name="small", bufs=4) as smp,
            tc.tile_pool(name="ps_sc", bufs=1, space="PSUM") as ps_sc,
            tc.tile_pool(name="ps_o", bufs=2, space="PSUM") as ps_o,
        ):
            for p in range(NPAIR):
                for g in range(2):
                    qt_sb = iop.tile([128, HALF], f32r, tag="qt")
                    nc.sync.dma_start(out=qt_sb[:], in_=qt2[p, g])
                    kt_sb = iop.tile([128, KT_W], f32r, tag="kt")
                    nc.sync.dma_start(out=kt_sb[:], in_=kt2[p, g])
                    va_sb = iop.tile([128, NCH * VW], f32r, tag="va")
                    nc.sync.dma_start(out=va_sb[:], in_=v1[2 * p, g])
                    vb_sb = iop.tile([128, NCH * VW], f32r, tag="vb")
                    nc.sync.dma_start(out=vb_sb[:], in_=v1[2 * p + 1, g])

                    for b in range(NBLK):
                        i = NBLK * g + b  # global block index
                        # window chunk c covers keys [256i-128+128c, +128);
                        # valid iff global key chunk 2i-1+c is in [0, 64)
                        cs = [c for c in range(4) if 0 <= 2 * i - 1 + c <= 63]
                        q0 = 256 * b

                        sc_a = ps_sc.tile([128, 1024], f32, tag="sca")
                        sc_b = ps_sc.tile([128, 1024], f32, tag="scb")
                        for c in cs:
                            kcol = 256 * b + 128 * c - 128 * (1 - g)
                            for h, sc in enumerate((sc_a, sc_b)):
                                nc.tensor.matmul(
                                    sc[:, 256 * c : 256 * c + 256],
                                    lhsT=kt_sb[
                                        64 * h : 64 * h + 64, kcol : kcol + 128
                                    ],
                                    rhs=qt_sb[
                                        64 * h : 64 * h + 64, q0 : q0 + 256
                                    ],
                                    start=True,
                                    stop=True,
                                    tile_position=(64 * h, 0),
                                )

                        et_a = etp.tile([128, 1024], f32r, tag="eta")
                        et_b = etp.tile([128, 1024], f32r, tag="etb")
                        lo, hi = 256 * cs[0], 256 * (cs[-1] + 1)
                        nc.scalar.activation(et_a[:, lo:hi], sc_a[:, lo:hi], Exp, scale=0.125)
                        nc.scalar.activation(et_b[:, lo:hi], sc_b[:, lo:hi], Exp, scale=0.125)

                        o_a = ps_o.tile([65, 256], f32, tag="oa")
                        o_b = ps_o.tile([65, 256], f32, tag="ob")
                        for o, et, v in ((o_a, et_a, va_sb), (o_b, et_b, vb_sb)):
                            for j, c in enumerate(cs):
                                lc = 2 * b - 1 + c + g  # chunk index in the half-local V tile
                                nc.tensor.matmul(
                                    o[:, :],
                                    lhsT=v[:, VW * lc : VW * lc + VW],
                                    rhs=et[:, 256 * c : 256 * c + 256],
                                    start=(j == 0),
                                    stop=(j == len(cs) - 1),
                                )

                        out_sb = outp.tile([128, 256], f32, tag="out")
                        for h, o in enumerate((o_a, o_b)):
                            r_sb = smp.tile([1, 256], f32, tag=f"r{h}")
                            nc.vector.reciprocal(r_sb[:, :], o[64:65, :])
                            rb_sb = smp.tile([64, 256], f32, tag=f"rb{h}")
                            nc.gpsimd.partition_broadcast(rb_sb[:, :], r_sb[:, :])
                            nc.vector.tensor_mul(
                                out=out_sb[64 * h : 64 * h + 64, :],
                                in0=o[0:64, :],
                                in1=rb_sb[:, :],
                            )
                        nc.sync.dma_start(
                            out=out2[p][:, 4096 * g + q0 : 4096 * g + q0 + 256],
                            in_=out_sb[:, :],
                        )
    nc.compile()
    return nc


def _prep_core(q, k, v, core):
    """Build the per-core input dict from full [64, T, D] arrays."""
    sl = slice(HPC * core, HPC * core + HPC)
    qt = np.ascontiguousarray(q[sl].transpose(0, 2, 1)).reshape(NPAIR, 128, T)
    qt2 = np.ascontiguousarray(np.stack([qt[:, :, 0:HALF], qt[:, :, HALF:T]], axis=1))
    kt = np.ascontiguousarray(k[sl].transpose(0, 2, 1)).reshape(NPAIR, 128, T)
    kt2 = np.ascontiguousarray(
        np.stack([kt[:, :, 0:KT_W], kt[:, :, T - KT_W : T]], axis=1)
    )
    vc = v[sl].reshape(HPC, 64, 128, D)
    v1full = np.concatenate([vc, np.ones((HPC, 64, 128, 1), np.float32)], axis=-1)
    v1g = np.stack([v1full[:, 0:NCH], v1full[:, 64 - NCH : 64]], axis=1)
    v1m = np.ascontiguousarray(v1g.transpose(0, 1, 3, 2, 4)).reshape(
        HPC, 2, 128, NCH * VW
    )
    return {"qt2": qt2, "kt2": kt2, "v1": v1m}


def kernel(query_layer, key_layer, value_layer):
    global LAST_EXEC_NS, LAST_RESULTS
    from concourse import bass_utils

    q = np.ascontiguousarray(np.asarray(query_layer, np.float32)).reshape(B * H, T, D)
    k = np.ascontiguousarray(np.asarray(key_layer, np.float32)).reshape(B * H, T, D)
    v = np.ascontiguousarray(np.asarray(value_layer, np.float32)).reshape(B * H, T, D)

    in_maps = [_prep_core(q, k, v, core) for core in range(NCORES)]
    nc = _build_nc()
    res = bass_utils.run_bass_kernel_spmd(
        nc, in_maps, core_ids=list(range(NCORES)), trace=TRACE
    )
    LAST_RESULTS = res
    LAST_EXEC_NS = res.exec_time_ns

    out = np.empty((B * H, T, D), np.float32)
    for core in range(NCORES):
        o2 = res.results[core]["out2"].reshape(NPAIR, 2, 64, T)
        out[HPC * core : HPC * core + HPC] = o2.transpose(0, 1, 3, 2).reshape(
            HPC, T, D
        )
    return out.reshape(B, H, T, D)
